# revision 30
# baseline (speedup 1.0000x reference)
"""Trainium2 Bass kernel for ContextualAttentionBlock.

Sharding: 8 cores, core c -> (batch b = c//2, query-half qh = c%2).
Each core computes, for its batch's 1024-token attention window:
  K/V projections for all 1024 tokens, Q for its 512 queries, RoPE,
  attention, out-proj, residual+RMSNorm1 -> h1 (512 tokens),
then SwiGLU FFN + residual + RMSNorm2 for 2048 tokens
  (512 attention-part tokens + 1536 "rest" tokens that skip attention).
All activations are kept feature-major ([feature, token]) so every matmul
contracts over the partition dim.  Weights and most attention activations
are bf16 (every matmul runs 1 cycle/row; bf16 DVE ops run 2x); RoPE runs
on bf16 copies staged off PSUM by the scalar engine.  The FFN runs
hidden-major over two 1024-token halves: gate/up/down weights stream once
per half, with silu(g)*u staged in bf16 SBUF tiles.
No collectives; the host shards inputs and reassembles the output.
"""

import numpy as np
import ml_dtypes

import concourse.bass as bass
import concourse.tile as tile
from concourse import bacc, mybir
from concourse.bass_utils import run_bass_kernel_spmd

F32 = mybir.dt.float32
F32R = mybir.dt.float32r
BF16 = mybir.dt.bfloat16
AF = mybir.ActivationFunctionType
OP = mybir.AluOpType

WIDTH = 1024
NT = 8              # width tiles of 128
HEADS = 16
HDIM = 64
LWIN = 1024         # attention window
LQ = 512            # queries per core
HID = 4096
NH = 32             # hidden tiles of 128
REST = 1536         # rest tokens per core
TOUT = LQ + REST    # 2048 ffn tokens per core, in 2 halves of 1024
EPS = 1e-6
ROPE_BASE = 10000.0
N_CORES = 8


def _r(ap):
    return ap.bitcast(mybir.dt.float32r)


def _emit(tc, A, out_ap):
    nc = tc.nc
    mm = nc.tensor.matmul

    xw_r = A["xw_t"].rearrange("(a p) t -> a p t", p=128)
    xq_r = A["xq_t"].rearrange("(a p) t -> a p t", p=128)
    xr_r = A["xr_t"].rearrange("(a p) t -> a p t", p=128)
    wqk4, wv4, wo4 = A["wqk4"], A["wv4"], A["wo4"]
    wgu4, wd4 = A["wgu4"], A["wd4"]
    out_r = out_ap.rearrange("(a p) t -> a p t", p=128)

    with tc.tile_pool(name="pc", bufs=1) as pc:
        # allocate persistent tiles first; DMAs are emitted in priority order
        cq = pc.tile([128, LQ], BF16, name="cq")
        sq = pc.tile([128, LQ], BF16, name="sq")
        ck = pc.tile([128, LWIN], BF16, name="ck")
        sk = pc.tile([128, LWIN], BF16, name="sk")
        g1 = pc.tile([128, NT], F32R, name="g1")
        g2 = pc.tile([128, NT], F32R, name="g2")
        onesF = pc.tile([128, 128], F32, name="onesF")
        onesK = pc.tile([128, 1], F32R, name="onesK")
        ones1 = pc.tile([1, 128], F32R, name="ones1")
        eps1 = pc.tile([1, 1], F32, name="eps1")
        ones64 = pc.tile([1, 64], F32R, name="ones64")
        # FFN token tiles: half 0 = [h1 | rest 0:512], half 1 = rest 512:1536
        tin0 = [pc.tile([128, 1024], BF16, name=f"tin0_{k}", tag=f"tin0_{k}")
                for k in range(NT)]
        tin1 = [pc.tile([128, 1024], BF16, name=f"tin1_{k}", tag=f"tin1_{k}")
                for k in range(NT)]

        # ---------------- Stage A: attention ----------------
        with tc.tile_pool(name="wc", bufs=1) as wc:
            with tc.tile_pool(name="pa", bufs=1) as pa, \
                 tc.tile_pool(name="wa", bufs=1) as wa, \
                 tc.tile_pool(name="psa", bufs=1, space="PSUM") as psa:
                ao = [pa.tile([128, LQ], BF16, name=f"ao_{i}", tag=f"ao{i}") for i in range(NT)]
                s_sb = [pa.tile([128, LQ], F32R, name=f"s_{m}", tag=f"s{m}") for m in range(NT)]
                xq = [pa.tile([128, LQ], BF16, name=f"xq_{k}", tag=f"xq_{k}") for k in range(NT)]
                xw = [pa.tile([128, LWIN], BF16, name=f"xw_{k}", tag=f"xw{k}") for k in range(NT)]

                # --- startup DMAs in critical-path order ---
                for k in range(NT):
                    nc.sync.dma_start(xq[k], xq_r[k])
                wqk_pf = []
                for side in range(2):
                    wb = wa.tile([128, NT, 256], BF16, name=f"wqkb_0_{side}", tag="wqk", bufs=4)
                    nc.sync.dma_start(wb, wqk4[:, side])
                    wqk_pf.append(wb)
                nc.sync.dma_start(cq, A["cos_q"])
                nc.sync.dma_start(sq, A["sin_q"])
                for k in range(4):
                    nc.sync.dma_start(xw[k], xw_r[k])
                nc.sync.dma_start(ck, A["cos_k"])
                nc.sync.dma_start(sk, A["sin_k"])
                for k in range(4, NT):
                    nc.sync.dma_start(xw[k], xw_r[k])
                wv_pf = wa.tile([128, NT, 256], BF16, name="wvb_0", tag="wv", bufs=1)
                nc.sync.dma_start(wv_pf, wv4[:, 0])
                nc.sync.dma_start(g1, A["g1"])
                nc.sync.dma_start(g2, A["g2"])
                nc.vector.memset(onesF, 1.0)
                nc.vector.tensor_copy(onesK, onesF[:, 0:1])
                nc.vector.tensor_copy(ones1, onesF[0:1, :])
                nc.vector.memset(eps1, EPS)
                nc.vector.tensor_copy(ones64, onesF[0:1, 0:64])
                onesKb = pc.tile([128, 1], BF16, name="onesKb")
                nc.vector.tensor_copy(onesKb, onesF[:, 0:1])

                def rope_from_psum(ps, dest, cos, sins, cpy, cps, t1):
                    # per 64-row head block: rows b..b+32 = even dims E,
                    # rows b+32..b+64 = odd dims O (head-contiguous perm).
                    # dest[E] = E*cos - O*sin ; dest[O] = O*cos + E*sin.
                    # ps is staged to a bf16 copy (scalar engine), the 32-row
                    # block swap is done by a local SBUF->SBUF DMA, and the
                    # block sign is baked into the host sin table, so the
                    # whole rotation is 3 full-width bf16 (2x) vector ops:
                    #   dest = cpy*cos + swap32(cpy)*sins
                    nc.vector.tensor_copy(cpy, ps)
                    for b in (0, 64):
                        nc.sync.dma_start(cps[b:b + 32, :], cpy[b + 32:b + 64, :])
                        nc.sync.dma_start(cps[b + 32:b + 64, :], cpy[b:b + 32, :])
                    nc.vector.tensor_mul(t1, cpy, cos)
                    nc.vector.tensor_mul(dest, cps, sins)
                    nc.vector.tensor_add(dest, dest, t1)

                for g in range(4):
                    # --- Q projection + RoPE (tiles 2g, 2g+1; heads 4g..4g+3) ---
                    q2 = []
                    wqk2 = []
                    for side, m in ((0, 2 * g), (1, 2 * g + 1)):
                        if g == 0:
                            wb = wqk_pf[side]
                        else:
                            wb = wa.tile([128, NT, 256], BF16, name=f"wqkb_{g}_{side}", tag="wqk", bufs=4)
                            nc.sync.dma_start(wb, wqk4[:, m])
                        wqk2.append(wb)
                        ps = psa.tile([128, LQ], F32, name=f"qps_{g}_{side}", tag="proj", bufs=2)
                        for k in range(NT):
                            mm(ps, wb[:, k, 0:128], xq[k], start=(k == 0), stop=(k == NT - 1))
                        qt = pa.tile([128, LQ], BF16, name=f"q_{g}_{side}",
                                     tag=("q0" if side == 0 else "q1"), bufs=2)
                        qc = pa.tile([128, LQ], BF16, name=f"qc_{g}_{side}", tag="rc", bufs=2)
                        qw = pa.tile([128, LQ], BF16, name=f"qw_{g}_{side}", tag="rw", bufs=2)
                        qs = pa.tile([128, LQ], BF16, name=f"qs_{g}_{side}", tag="rs", bufs=2)
                        rope_from_psum(ps, qt, cq, sq, qc, qw, qs)
                        q2.append(qt)

                    # --- K projection + RoPE ---
                    k2 = []
                    for side, m in ((0, 2 * g), (1, 2 * g + 1)):
                        wb = wqk2[side]
                        kt_sb = pa.tile([128, LWIN], BF16, name=f"k_{g}_{side}",
                                        tag=("k0" if side == 0 else "k1"), bufs=2)
                        for ch in range(2):
                            ps = psa.tile([128, 512], F32, name=f"kps_{g}_{side}_{ch}", tag="proj", bufs=2)
                            for k in range(NT):
                                mm(ps, wb[:, k, 128:256], xw[k][:, ch * 512:(ch + 1) * 512],
                                   start=(k == 0), stop=(k == NT - 1))
                            kc = pa.tile([128, 512], BF16, name=f"kc_{g}_{side}_{ch}", tag="rc", bufs=2)
                            kw = pa.tile([128, 512], BF16, name=f"kw_{g}_{side}_{ch}", tag="rw", bufs=2)
                            ks = pa.tile([128, 512], BF16, name=f"ks_{g}_{side}_{ch}", tag="rs", bufs=2)
                            rope_from_psum(ps, kt_sb[:, ch * 512:(ch + 1) * 512],
                                           ck[:, ch * 512:(ch + 1) * 512],
                                           sk[:, ch * 512:(ch + 1) * 512], kc, kw, ks)
                        k2.append(kt_sb)

                    # --- V projection (token-major, 65-col per head with ones col) ---
                    if g == 0:
                        wvb = wv_pf
                    else:
                        wvb = wa.tile([128, NT, 256], BF16, name=f"wvb_{g}", tag="wv", bufs=1)
                        nc.sync.dma_start(wvb, wv4[:, g])
                    vg = [pa.tile([128, 4 * 65], BF16, name=f"v_{g}_{kt}", tag=f"v{kt}", bufs=2)
                          for kt in range(NT)]
                    for kt in range(NT):
                        psv = psa.tile([128, 256], F32, name=f"vps_{g}_{kt}", tag="proj", bufs=2)
                        for k in range(NT):
                            mm(psv, xw[k][:, kt * 128:(kt + 1) * 128], wvb[:, k, :],
                               start=(k == 0), stop=(k == NT - 1))
                        v3 = vg[kt].rearrange("p (h c) -> p h c", c=65)
                        nc.vector.tensor_copy(v3[:, :, 64:65],
                                              onesF[:, 0:1].unsqueeze(1).broadcast_to([128, 4, 1]))
                        nc.vector.tensor_copy(v3[:, :, 0:64], psv.rearrange("p (h c) -> p h c", c=64))

                    # --- attention per head pair ---
                    for p2 in range(2):
                        vac = [psa.tile([65, 512], F32, name=f"vac_{g}_{p2}_{jj}", tag="vac", bufs=4)
                               for jj in range(2)]
                        for kt in range(NT):
                            for jj in range(2):
                                j = 2 * p2 + jj
                                sc = psa.tile([128, 512], F32, name=f"sc_{g}_{p2}_{kt}_{jj}",
                                              tag="sc", bufs=2)
                                mm(sc, k2[p2][64 * jj:64 * (jj + 1), kt * 128:(kt + 1) * 128],
                                   q2[p2][64 * jj:64 * (jj + 1), :],
                                   start=True, stop=True, tile_position=(64 * jj, 0))
                                at = pa.tile([128, 512], BF16, name=f"at_{g}_{p2}_{kt}_{jj}",
                                             tag="at", bufs=4)
                                nc.scalar.activation(at, sc, AF.Exp, scale=0.125)
                                mm(vac[jj], vg[kt][:, j * 65:(j + 1) * 65], at,
                                   start=(kt == 0), stop=(kt == NT - 1))
                        # normalize pair -> attention out tile i (heads 2i, 2i+1)
                        rr = pa.tile([1, 1024], F32R, name=f"rr_{g}_{p2}", tag="rr", bufs=2)
                        with nc.allow_low_precision(reason="tf32 softmax denom"):
                            nc.vector.reciprocal(rr[0:1, 0:512], vac[0][64:65, :])
                            nc.vector.reciprocal(rr[0:1, 512:1024], vac[1][64:65, :])
                        bc0 = psa.tile([64, 512], F32, name=f"bca_{g}_{p2}_0", tag="vac", bufs=4)
                        mm(bc0, _r(ones64), _r(rr[0:1, 0:512]))
                        bc1 = psa.tile([64, 512], F32, name=f"bca_{g}_{p2}_1", tag="vac", bufs=4)
                        mm(bc1, _r(ones64), _r(rr[0:1, 512:1024]))
                        bcs = pa.tile([128, 512], F32R, name=f"bcs_{g}_{p2}", tag="bcs", bufs=1)
                        nc.vector.tensor_copy(bcs[0:64, :], bc0)
                        nc.vector.tensor_copy(bcs[64:128, :], bc1)
                        i = 2 * g + p2
                        nc.vector.tensor_mul(ao[i][0:64, :], vac[0][0:64, :], bcs[0:64, :])
                        nc.vector.tensor_mul(ao[i][64:128, :], vac[1][0:64, :], bcs[64:128, :])

                # --- prefetch FFN inputs and first weight tiles ---
                for k in range(NT):
                    nc.sync.dma_start(tin0[k][:, LQ:1024], xr_r[k][:, 0:LQ])
                    nc.sync.dma_start(tin1[k], xr_r[k][:, LQ:REST])
                ffn_pf = []
                for hm in range(2):
                    wgub = wc.tile([128, NT, 256], BF16, name=f"wgub_0_{hm}", tag="wgu", bufs=6)
                    nc.sync.dma_start(wgub, wgu4[:, hm])
                    ffn_pf.append(wgub)

                # ---------------- Stage B: out-proj + RMSNorm1 -> h1 ----------------
                ssp = psa.tile([1, 512], F32, name="ssp", tag="sc", bufs=2)
                for m in range(NT):
                    wb = wa.tile([128, NT, 128], BF16, name=f"wob_{m}", tag="wqk", bufs=4)
                    nc.sync.dma_start(wb, wo4[:, m])
                    yp = psa.tile([128, LQ], F32, name=f"yps_{m}", tag="proj", bufs=2)
                    for k in range(NT):
                        mm(yp, wb[:, k, :], ao[k], start=(k == 0), stop=(k == NT - 1))
                    nc.vector.tensor_add(s_sb[m], xq[m], yp)
                    sqt = pa.tile([128, LQ], F32R, name=f"sq1_{m}", tag="sq", bufs=2)
                    nc.vector.tensor_mul(sqt, s_sb[m], s_sb[m])
                    mm(ssp, _r(onesK), _r(sqt), start=(m == 0), stop=(m == NT - 1))
                row = pa.tile([1, 512], F32R, name="row1", tag="row", bufs=2)
                nc.scalar.activation(row, ssp, AF.Sqrt, scale=1.0 / WIDTH, bias=eps1)
                with nc.allow_low_precision(reason="tf32 rstd"):
                    nc.vector.reciprocal(row, row)
                bcn = psa.tile([128, 512], F32, name="bcn", tag="vac", bufs=4)
                mm(bcn, _r(ones1), _r(row))
                bcn_s = pa.tile([128, 512], F32R, name="bcn_s", tag="bcs", bufs=1)
                nc.vector.tensor_copy(bcn_s, bcn)
                for m in range(NT):
                    nc.vector.scalar_tensor_tensor(tin0[m][:, 0:LQ], s_sb[m],
                                                   g1[:, m:m + 1], bcn_s,
                                                   op0=OP.mult, op1=OP.mult)

            # ---------------- Stage C: SwiGLU FFN + RMSNorm2 ----------------
            # hidden-major: per 1024-token half, stream gate/up weights once
            # over all 32 hidden tiles producing F = silu(g)*u (bf16), then
            # stream the down weights once over the 8 output tiles.
            with tc.tile_pool(name="pcn", bufs=1) as pcn, \
                 tc.tile_pool(name="psc", bufs=1, space="PSUM") as psc:
                for hi, half in enumerate((1, 0)):  # half 1 first: it needs only
                    # DMA'd inputs, so the PE never waits on the stage-B norm chain
                    tin = tin0 if half == 0 else tin1
                    F_t = [[pcn.tile([128, 512], BF16, name=f"F_{half}_{sc}_{h}",
                                     tag=f"F{sc}_{h}") for h in range(NH)]
                           for sc in range(2)]
                    for hm in range(NH):
                        if hi == 0 and hm < 2:
                            wgub = ffn_pf[hm]
                        else:
                            wgub = wc.tile([128, NT, 256], BF16, name=f"wgub_{half}_{hm}",
                                           tag="wgu", bufs=6)
                            nc.sync.dma_start(wgub, wgu4[:, hm])
                        for sc in range(2):
                            gp = psc.tile([128, 512], F32, name=f"gp_{half}_{hm}_{sc}",
                                          tag="gu", bufs=4)
                            up = psc.tile([128, 512], F32, name=f"up_{half}_{hm}_{sc}",
                                          tag="gu", bufs=4)
                            for k in range(NT):
                                mm(gp, wgub[:, k, 0:128], tin[k][:, sc * 512:(sc + 1) * 512],
                                   start=(k == 0), stop=(k == NT - 1))
                            for k in range(NT):
                                mm(up, wgub[:, k, 128:256], tin[k][:, sc * 512:(sc + 1) * 512],
                                   start=(k == 0), stop=(k == NT - 1))
                            sg = pcn.tile([128, 512], F32R, name=f"sg_{half}_{hm}_{sc}",
                                          tag="sg", bufs=2)
                            nc.scalar.activation(sg, gp, AF.Silu)
                            nc.vector.tensor_mul(F_t[sc][hm], sg, up)
                    # down-proj + residual + RMSNorm2
                    s2l = [pcn.tile([128, 1024], BF16, name=f"s2_{half}_{m}", tag=f"s2_{m}")
                           for m in range(NT)]
                    ssp2 = [psc.tile([1, 512], F32, name=f"ssp2_{half}_{sc}", tag="st", bufs=2)
                            for sc in range(2)]
                    for m in range(NT):
                        wdb = wc.tile([128, NH, 128], BF16, name=f"wdb_{half}_{m}",
                                      tag="wd", bufs=3)
                        nc.sync.dma_start(wdb, wd4[:, m])
                        for sc in range(2):
                            dp = psc.tile([128, 512], F32, name=f"dp_{half}_{m}_{sc}",
                                          tag="d", bufs=2)
                            for hm in range(NH):
                                mm(dp, wdb[:, hm, :], F_t[sc][hm],
                                   start=(hm == 0), stop=(hm == NH - 1))
                            sl = s2l[m][:, sc * 512:(sc + 1) * 512]
                            nc.vector.tensor_add(sl, tin[m][:, sc * 512:(sc + 1) * 512], dp)
                            sqt = pcn.tile([128, 512], BF16, name=f"sq2_{half}_{m}_{sc}",
                                           tag="sq2", bufs=2)
                            nc.vector.tensor_mul(sqt, sl, sl)
                            mm(ssp2[sc], onesKb, sqt, start=(m == 0), stop=(m == NT - 1))
                    for sc in range(2):
                        row2 = pcn.tile([1, 512], F32R, name=f"row2_{half}_{sc}", tag="row2", bufs=2)
                        nc.scalar.activation(row2, ssp2[sc], AF.Sqrt, scale=1.0 / WIDTH, bias=eps1)
                        with nc.allow_low_precision(reason="tf32 rstd"):
                            nc.vector.reciprocal(row2, row2)
                        bc2 = psc.tile([128, 512], F32, name=f"bc2_{half}_{sc}", tag="d", bufs=2)
                        mm(bc2, _r(ones1), _r(row2))
                        bc2_s = pcn.tile([128, 512], F32R, name=f"bc2s_{half}_{sc}",
                                         tag="bc2s", bufs=2)
                        nc.vector.tensor_copy(bc2_s, bc2)
                        for m in range(NT):
                            ot = pcn.tile([128, 512], F32R, name=f"ot_{half}_{sc}_{m}",
                                          tag="ot", bufs=2)
                            nc.vector.scalar_tensor_tensor(ot, s2l[m][:, sc * 512:(sc + 1) * 512],
                                                           g2[:, m:m + 1], bc2_s,
                                                           op0=OP.mult, op1=OP.mult)
                            nc.sync.dma_start(
                                out_r[m][:, half * 1024 + sc * 512:half * 1024 + (sc + 1) * 512], ot)


_INPUT_SPECS = [
    ("xw_t", [WIDTH, LWIN], BF16),
    ("xq_t", [WIDTH, LQ], BF16),
    ("xr_t", [WIDTH, REST], BF16),
    ("wqk4", [128, NT, NT, 256], BF16),
    ("wv4", [128, 4, NT, 256], BF16),
    ("wo4", [128, NT, NT, 128], BF16),
    ("wgu4", [128, NH, NT, 256], BF16),
    ("wd4", [128, NT, NH, 128], BF16),
    ("cos_q", [128, LQ], BF16),
    ("sin_q", [128, LQ], BF16),
    ("cos_k", [128, LWIN], BF16),
    ("sin_k", [128, LWIN], BF16),
    ("g1", [128, NT], F32R),
    ("g2", [128, NT], F32R),
]


def build_program(reps=1):
    nc = bacc.Bacc("TRN2", target_bir_lowering=False, debug=False, num_devices=N_CORES)
    A = {name: nc.dram_tensor(name, shape, dt, kind="ExternalInput").ap()
         for name, shape, dt in _INPUT_SPECS}
    out_ap = nc.dram_tensor("out_t", [WIDTH, TOUT], F32R, kind="ExternalOutput").ap()
    with tile.TileContext(nc) as tc:
        for _ in range(reps):
            _emit(tc, A, out_ap)
    nc.compile()
    return nc


def _w4(w_t, a, c):
    """[K, M] weight (contract-major) -> [128, M/c-tiles, K/128, c] bf16 blocks."""
    k, m = w_t.shape
    return np.ascontiguousarray(
        w_t.reshape(k // 128, 128, m // c, c).transpose(1, 2, 0, 3)
        .astype(ml_dtypes.bfloat16))


def make_in_maps(x, w_qkv, w_out, g_norm1, g_norm2, w_gate, w_up, w_down):
    f32 = np.float32
    bf16 = ml_dtypes.bfloat16
    x = np.asarray(x, f32)
    w_qkv = np.asarray(w_qkv, f32)
    # head-contiguous de-interleave: within head h, even dims first then odd:
    # new row h*64+j -> old h*64+2j ; new row h*64+32+j -> old h*64+2j+1
    perm = np.empty(WIDTH, np.int64)
    for h in range(HEADS):
        j = np.arange(32)
        perm[h * 64 + j] = h * 64 + 2 * j
        perm[h * 64 + 32 + j] = h * 64 + 2 * j + 1
    wq = w_qkv[0:WIDTH][perm]
    wk = w_qkv[WIDTH:2 * WIDTH][perm]
    wv = w_qkv[2 * WIDTH:3 * WIDTH]

    inv_freq = (1.0 / (ROPE_BASE ** (np.arange(0, HDIM, 2, dtype=np.float64) / HDIM)))

    def tab(pos):
        # cos straight; sin with the rotation sign baked in per 32-row block
        # (rows r with r%64<32 hold even dims E: dest_E = E*cos - O*sin).
        fr = np.outer(inv_freq, pos.astype(np.float64))  # [32, T]
        s = np.sin(fr)
        return (np.tile(np.cos(fr), (4, 1)).astype(bf16),
                np.ascontiguousarray(
                    np.concatenate([-s, s, -s, s], axis=0)).astype(bf16))

    cos_k, sin_k = tab(np.arange(LWIN))
    wq4 = _w4(np.ascontiguousarray(wq.T), NT, 128)
    wk4 = _w4(np.ascontiguousarray(wk.T), NT, 128)
    wg4 = _w4(np.ascontiguousarray(np.asarray(w_gate, f32).T), NH, 128)
    wu4 = _w4(np.ascontiguousarray(np.asarray(w_up, f32).T), NH, 128)
    common = {
        "wqk4": np.ascontiguousarray(np.concatenate([wq4, wk4], axis=3)),
        "wv4": _w4(np.ascontiguousarray(wv.T), 4, 256),
        "wo4": _w4(np.ascontiguousarray(np.asarray(w_out, f32).T), NT, 128),
        "wgu4": np.ascontiguousarray(np.concatenate([wg4, wu4], axis=3)),
        "wd4": _w4(np.ascontiguousarray(np.asarray(w_down, f32).T), NT, 128),
        "cos_k": cos_k,
        "sin_k": sin_k,
        "g1": np.ascontiguousarray(np.asarray(g_norm1, f32).reshape(NT, 128).T),
        "g2": np.ascontiguousarray(np.asarray(g_norm2, f32).reshape(NT, 128).T),
    }
    in_maps = []
    for c in range(N_CORES):
        b, qh = c // 2, c % 2
        cos_q, sin_q = tab(np.arange(qh * LQ, (qh + 1) * LQ))
        m = dict(common)
        m["xw_t"] = np.ascontiguousarray(x[b, :LWIN].T).astype(bf16)
        m["xq_t"] = np.ascontiguousarray(x[b, qh * LQ:(qh + 1) * LQ].T).astype(bf16)
        m["xr_t"] = np.ascontiguousarray(
            x[b, LWIN + qh * REST:LWIN + (qh + 1) * REST].T).astype(bf16)
        m["cos_q"] = cos_q
        m["sin_q"] = sin_q
        in_maps.append(m)
    return in_maps


def assemble_output(results):
    out = np.empty((4, 4096, WIDTH), np.float32)
    for c in range(N_CORES):
        b, qh = c // 2, c % 2
        o = results[c]["out_t"]
        out[b, qh * LQ:(qh + 1) * LQ] = o[:, :LQ].T
        out[b, LWIN + qh * REST:LWIN + (qh + 1) * REST] = o[:, LQ:].T
    return out


_CACHE = {}


def kernel(x, w_qkv, w_out, g_norm1, g_norm2, w_gate, w_up, w_down):
    if "nc" not in _CACHE:
        _CACHE["nc"] = build_program()
    nc = _CACHE["nc"]
    in_maps = make_in_maps(x, w_qkv, w_out, g_norm1, g_norm2, w_gate, w_up, w_down)
    res = run_bass_kernel_spmd(nc, in_maps, list(range(N_CORES))).results
    return assemble_output(res)


# revision 34
# speedup vs baseline: 1.1082x; 1.1082x over previous
"""Trainium2 Bass kernel for ContextualAttentionBlock.

Sharding: 8 cores, core c -> (batch b = c//2, query-half qh = c%2).
Each core computes, for its batch's 1024-token attention window:
  K/V projections for all 1024 tokens, Q for its 512 queries, RoPE,
  attention, out-proj, residual+RMSNorm1 -> h1 (512 tokens),
then SwiGLU FFN + residual + RMSNorm2 for 2048 tokens
  (512 attention-part tokens + 1536 "rest" tokens that skip attention).
All activations are kept feature-major ([feature, token]) so every matmul
contracts over the partition dim.  Weights and most attention activations
are bf16 (every matmul runs 1 cycle/row; bf16 DVE ops run 2x); RoPE runs
on bf16 copies staged off PSUM by the scalar engine.  The FFN runs
hidden-major over two 1024-token halves: gate/up/down weights stream once
per half, with silu(g)*u staged in bf16 SBUF tiles.
No collectives; the host shards inputs and reassembles the output.
"""

import numpy as np
import ml_dtypes

import concourse.bass as bass
import concourse.tile as tile
from concourse import bacc, mybir
from concourse.bass_utils import run_bass_kernel_spmd

F32 = mybir.dt.float32
F32R = mybir.dt.float32r
BF16 = mybir.dt.bfloat16
AF = mybir.ActivationFunctionType
OP = mybir.AluOpType

WIDTH = 1024
NT = 8              # width tiles of 128
HEADS = 16
HDIM = 64
LWIN = 1024         # attention window
LQ = 512            # queries per core
HID = 4096
NH = 32             # hidden tiles of 128
REST = 1536         # rest tokens per core
TOUT = LQ + REST    # 2048 ffn tokens per core, in 2 halves of 1024
EPS = 1e-6
ROPE_BASE = 10000.0
N_CORES = 8


def _r(ap):
    return ap.bitcast(mybir.dt.float32r)


def _emit(tc, A, out_ap):
    nc = tc.nc
    mm = nc.tensor.matmul

    xw_r = A["xw_t"].rearrange("(a p) t -> a p t", p=128)
    xq_r = A["xq_t"].rearrange("(a p) t -> a p t", p=128)
    xr_r = A["xr_t"].rearrange("(a p) t -> a p t", p=128)
    wqk4, wv4, wo4 = A["wqk4"], A["wv4"], A["wo4"]
    wgu4, wd4 = A["wgu4"], A["wd4"]
    out_r = out_ap.rearrange("(a p) t -> a p t", p=128)

    with tc.tile_pool(name="pc", bufs=1) as pc:
        # allocate persistent tiles first; DMAs are emitted in priority order
        cq = pc.tile([128, LQ], BF16, name="cq")
        sq = pc.tile([128, LQ], BF16, name="sq")
        ck = pc.tile([128, LWIN], BF16, name="ck")
        sk = pc.tile([128, LWIN], BF16, name="sk")
        g1 = pc.tile([128, NT], F32R, name="g1")
        g2 = pc.tile([128, NT], F32R, name="g2")
        onesF = pc.tile([128, 128], BF16, name="onesF")
        onesK = pc.tile([128, 1], F32R, name="onesK")
        ones1 = pc.tile([1, 128], F32R, name="ones1")
        eps1 = pc.tile([1, 1], F32, name="eps1")
        ones64 = pc.tile([1, 64], F32R, name="ones64")
        # FFN token tiles: half 0 = [h1 | rest 0:512], half 1 = rest 512:1536
        tin0 = [pc.tile([128, 1024], BF16, name=f"tin0_{k}", tag=f"tin0_{k}")
                for k in range(NT)]
        tin1 = [pc.tile([128, 1024], BF16, name=f"tin1_{k}", tag=f"tin1_{k}")
                for k in range(NT)]

        # ---------------- Stage A: attention ----------------
        with tc.tile_pool(name="wc", bufs=1) as wc:
            with tc.tile_pool(name="pa", bufs=1) as pa, \
                 tc.tile_pool(name="wa", bufs=1) as wa, \
                 tc.tile_pool(name="psa", bufs=1, space="PSUM") as psa:
                ao = [pa.tile([128, LQ], BF16, name=f"ao_{i}", tag=f"ao{i}") for i in range(NT)]
                s_sb = [pa.tile([128, LQ], BF16, name=f"s_{m}", tag=f"s{m}") for m in range(NT)]
                xq = [pa.tile([128, LQ], BF16, name=f"xq_{k}", tag=f"xq_{k}") for k in range(NT)]
                xw = [pa.tile([128, LWIN], BF16, name=f"xw_{k}", tag=f"xw{k}") for k in range(NT)]

                # --- startup DMAs in critical-path order ---
                for k in range(NT):
                    nc.sync.dma_start(xq[k], xq_r[k])
                wqk_pf = []
                for side in range(2):
                    wb = wa.tile([128, NT, 256], BF16, name=f"wqkb_0_{side}", tag="wqk", bufs=3)
                    nc.sync.dma_start(wb, wqk4[:, side])
                    wqk_pf.append(wb)
                nc.sync.dma_start(cq, A["cos_q"])
                nc.sync.dma_start(sq, A["sin_q"])
                for k in range(4):
                    nc.sync.dma_start(xw[k], xw_r[k])
                nc.sync.dma_start(ck, A["cos_k"])
                nc.sync.dma_start(sk, A["sin_k"])
                for k in range(4, NT):
                    nc.sync.dma_start(xw[k], xw_r[k])
                wv_pf = wa.tile([128, NT, 256], BF16, name="wvb_0", tag="wv", bufs=1)
                nc.sync.dma_start(wv_pf, wv4[:, 0])
                nc.sync.dma_start(g1, A["g1"])
                nc.sync.dma_start(g2, A["g2"])
                nc.vector.memset(onesF, 1.0)
                nc.vector.tensor_copy(onesK, onesF[:, 0:1])
                nc.vector.tensor_copy(ones1, onesF[0:1, :])
                nc.vector.memset(eps1, EPS)
                nc.vector.tensor_copy(ones64, onesF[0:1, 0:64])
                onesKb = pc.tile([128, 1], BF16, name="onesKb")
                nc.vector.tensor_copy(onesKb, onesF[:, 0:1])

                def rope_from_psum(ps, dest, cos, sins, cpy, cps, t1):
                    # per 64-row head block: rows b..b+32 = even dims E,
                    # rows b+32..b+64 = odd dims O (head-contiguous perm).
                    # dest[E] = E*cos - O*sin ; dest[O] = O*cos + E*sin.
                    # ps is staged to a bf16 copy (scalar engine), the 32-row
                    # block swap is done by a local SBUF->SBUF DMA, and the
                    # block sign is baked into the host sin table, so the
                    # whole rotation is 3 full-width bf16 (2x) vector ops:
                    #   dest = cpy*cos + swap32(cpy)*sins
                    nc.vector.tensor_copy(cpy, ps)
                    for b in (0, 64):
                        nc.sync.dma_start(cps[b:b + 32, :], cpy[b + 32:b + 64, :])
                        nc.sync.dma_start(cps[b + 32:b + 64, :], cpy[b:b + 32, :])
                    nc.vector.tensor_mul(t1, cpy, cos)
                    nc.vector.tensor_mul(dest, cps, sins)
                    nc.vector.tensor_add(dest, dest, t1)

                for g in range(4):
                    # --- Q projection + RoPE (tiles 2g, 2g+1; heads 4g..4g+3) ---
                    q2 = []
                    wqk2 = []
                    for side, m in ((0, 2 * g), (1, 2 * g + 1)):
                        if g == 0:
                            wb = wqk_pf[side]
                        else:
                            wb = wa.tile([128, NT, 256], BF16, name=f"wqkb_{g}_{side}", tag="wqk", bufs=3)
                            nc.sync.dma_start(wb, wqk4[:, m])
                        wqk2.append(wb)
                        ps = psa.tile([128, LQ], F32, name=f"qps_{g}_{side}", tag="proj", bufs=2)
                        for k in range(NT):
                            mm(ps, wb[:, k, 0:128], xq[k], start=(k == 0), stop=(k == NT - 1))
                        qt = pa.tile([128, LQ], BF16, name=f"q_{g}_{side}",
                                     tag=("q0" if side == 0 else "q1"), bufs=2)
                        qc = pa.tile([128, LQ], BF16, name=f"qc_{g}_{side}", tag="rc", bufs=2)
                        qw = pa.tile([128, LQ], BF16, name=f"qw_{g}_{side}", tag="rw", bufs=2)
                        qs = pa.tile([128, LQ], BF16, name=f"qs_{g}_{side}", tag="rs", bufs=2)
                        rope_from_psum(ps, qt, cq, sq, qc, qw, qs)
                        q2.append(qt)

                    # --- K projection + RoPE ---
                    k2 = []
                    for side, m in ((0, 2 * g), (1, 2 * g + 1)):
                        wb = wqk2[side]
                        kt_sb = pa.tile([128, LWIN], BF16, name=f"k_{g}_{side}",
                                        tag=("k0" if side == 0 else "k1"), bufs=2)
                        for ch in range(2):
                            ps = psa.tile([128, 512], F32, name=f"kps_{g}_{side}_{ch}", tag="proj", bufs=2)
                            for k in range(NT):
                                mm(ps, wb[:, k, 128:256], xw[k][:, ch * 512:(ch + 1) * 512],
                                   start=(k == 0), stop=(k == NT - 1))
                            kc = pa.tile([128, 512], BF16, name=f"kc_{g}_{side}_{ch}", tag="rc", bufs=2)
                            kw = pa.tile([128, 512], BF16, name=f"kw_{g}_{side}_{ch}", tag="rw", bufs=2)
                            ks = pa.tile([128, 512], BF16, name=f"ks_{g}_{side}_{ch}", tag="rs", bufs=2)
                            rope_from_psum(ps, kt_sb[:, ch * 512:(ch + 1) * 512],
                                           ck[:, ch * 512:(ch + 1) * 512],
                                           sk[:, ch * 512:(ch + 1) * 512], kc, kw, ks)
                        k2.append(kt_sb)

                    # --- V projection (token-major, 65-col per head with ones col) ---
                    if g == 0:
                        wvb = wv_pf
                    else:
                        wvb = wa.tile([128, NT, 256], BF16, name=f"wvb_{g}", tag="wv", bufs=1)
                        nc.sync.dma_start(wvb, wv4[:, g])
                    vg = [pa.tile([128, 4 * 65], BF16, name=f"v_{g}_{kt}", tag=f"v{kt}", bufs=2)
                          for kt in range(NT)]
                    for kt in range(NT):
                        psv = psa.tile([128, 256], F32, name=f"vps_{g}_{kt}", tag="proj", bufs=2)
                        for k in range(NT):
                            mm(psv, xw[k][:, kt * 128:(kt + 1) * 128], wvb[:, k, :],
                               start=(k == 0), stop=(k == NT - 1))
                        v3 = vg[kt].rearrange("p (h c) -> p h c", c=65)
                        nc.vector.tensor_copy(v3[:, :, 64:65],
                                              onesF[:, 0:1].unsqueeze(1).broadcast_to([128, 4, 1]))
                        nc.vector.tensor_copy(v3[:, :, 0:64], psv.rearrange("p (h c) -> p h c", c=64))

                    # --- attention per head pair ---
                    for p2 in range(2):
                        vac = [psa.tile([65, 512], F32, name=f"vac_{g}_{p2}_{jj}", tag="vac", bufs=4)
                               for jj in range(2)]
                        ats = [[None] * 2 for _ in range(NT)]
                        for kt in range(NT):
                            for jj in range(2):
                                sc = psa.tile([128, 512], F32, name=f"sc_{g}_{p2}_{kt}_{jj}",
                                              tag="sc", bufs=2)
                                mm(sc, k2[p2][64 * jj:64 * (jj + 1), kt * 128:(kt + 1) * 128],
                                   q2[p2][64 * jj:64 * (jj + 1), :],
                                   start=True, stop=True, tile_position=(64 * jj, 0))
                                at = pa.tile([128, 512], BF16, name=f"at_{g}_{p2}_{kt}_{jj}",
                                             tag=f"at{jj}", bufs=8)
                                nc.scalar.activation(at, sc, AF.Exp, scale=0.125)
                                ats[kt][jj] = at
                        # contiguous accumulation groups (no interleaved matmuls)
                        for jj in range(2):
                            j = 2 * p2 + jj
                            for kt in range(NT):
                                mm(vac[jj], vg[kt][:, j * 65:(j + 1) * 65], ats[kt][jj],
                                   start=(kt == 0), stop=(kt == NT - 1))
                        # normalize pair -> attention out tile i (heads 2i, 2i+1)
                        rr = pa.tile([1, 1024], F32R, name=f"rr_{g}_{p2}", tag="rr", bufs=2)
                        with nc.allow_low_precision(reason="tf32 softmax denom"):
                            nc.vector.reciprocal(rr[0:1, 0:512], vac[0][64:65, :])
                            nc.vector.reciprocal(rr[0:1, 512:1024], vac[1][64:65, :])
                        bc0 = psa.tile([64, 512], F32, name=f"bca_{g}_{p2}_0", tag="vac", bufs=4)
                        mm(bc0, _r(ones64), _r(rr[0:1, 0:512]))
                        bc1 = psa.tile([64, 512], F32, name=f"bca_{g}_{p2}_1", tag="vac", bufs=4)
                        mm(bc1, _r(ones64), _r(rr[0:1, 512:1024]))
                        bcs = pa.tile([128, 512], F32, name=f"bcs_{g}_{p2}", tag="bcs", bufs=1)
                        nc.vector.tensor_copy(bcs[0:64, :], bc0)
                        nc.vector.tensor_copy(bcs[64:128, :], bc1)
                        i = 2 * g + p2
                        nc.vector.tensor_mul(ao[i][0:64, :], vac[0][0:64, :], bcs[0:64, :])
                        nc.vector.tensor_mul(ao[i][64:128, :], vac[1][0:64, :], bcs[64:128, :])

                # --- prefetch FFN inputs and first weight tiles ---
                for k in range(NT):
                    nc.sync.dma_start(tin0[k][:, LQ:1024], xr_r[k][:, 0:LQ])
                    nc.sync.dma_start(tin1[k], xr_r[k][:, LQ:REST])
                ffn_pf = []
                for hm in range(2):
                    wgub = wc.tile([128, NT, 256], BF16, name=f"wgub_0_{hm}", tag="wgu", bufs=6)
                    nc.sync.dma_start(wgub, wgu4[:, hm])
                    ffn_pf.append(wgub)

                # ---------------- Stage B: out-proj + RMSNorm1 -> h1 ----------------
                ssp = psa.tile([1, 512], F32, name="ssp", tag="sc", bufs=2)
                sqts = []
                for m in range(NT):
                    wb = wa.tile([128, NT, 128], BF16, name=f"wob_{m}", tag="wqk", bufs=3)
                    nc.sync.dma_start(wb, wo4[:, m])
                    yp = psa.tile([128, LQ], F32, name=f"yps_{m}", tag="proj", bufs=2)
                    for k in range(NT):
                        mm(yp, wb[:, k, :], ao[k], start=(k == 0), stop=(k == NT - 1))
                    nc.vector.tensor_add(s_sb[m], xq[m], yp)
                    sqt = pa.tile([128, LQ], BF16, name=f"sq1_{m}", tag="sq", bufs=8)
                    nc.vector.tensor_mul(sqt, s_sb[m], s_sb[m])
                    sqts.append(sqt)
                for m in range(NT):
                    mm(ssp, onesKb, sqts[m], start=(m == 0), stop=(m == NT - 1))
                row = pa.tile([1, 512], F32R, name="row1", tag="row", bufs=2)
                nc.scalar.activation(row, ssp, AF.Sqrt, scale=1.0 / WIDTH, bias=eps1)
                with nc.allow_low_precision(reason="tf32 rstd"):
                    nc.vector.reciprocal(row, row)
                bcn = psa.tile([128, 512], F32, name="bcn", tag="vac", bufs=4)
                mm(bcn, _r(ones1), _r(row))
                bcn_s = pa.tile([128, 512], F32R, name="bcn_s", tag="bcs", bufs=1)
                nc.vector.tensor_copy(bcn_s, bcn)
                for m in range(NT):
                    nc.vector.scalar_tensor_tensor(tin0[m][:, 0:LQ], s_sb[m],
                                                   g1[:, m:m + 1], bcn_s,
                                                   op0=OP.mult, op1=OP.mult)

            # ---------------- Stage C: SwiGLU FFN + RMSNorm2 ----------------
            # hidden-major: per 1024-token half, stream gate/up weights once
            # over all 32 hidden tiles producing F = silu(g)*u (bf16), then
            # stream the down weights once over the 8 output tiles.
            with tc.tile_pool(name="pcn", bufs=1) as pcn, \
                 tc.tile_pool(name="psc", bufs=1, space="PSUM") as psc:
                for hi, half in enumerate((1, 0)):  # half 1 first: it needs only
                    # DMA'd inputs, so the PE never waits on the stage-B norm chain
                    tin = tin0 if half == 0 else tin1
                    F_t = [[pcn.tile([128, 512], BF16, name=f"F_{half}_{sc}_{h}",
                                     tag=f"F{sc}_{h}") for h in range(NH)]
                           for sc in range(2)]
                    for hm in range(NH):
                        if hi == 0 and hm < 2:
                            wgub = ffn_pf[hm]
                        else:
                            wgub = wc.tile([128, NT, 256], BF16, name=f"wgub_{half}_{hm}",
                                           tag="wgu", bufs=6)
                            nc.sync.dma_start(wgub, wgu4[:, hm])
                        for sc in range(2):
                            gp = psc.tile([128, 512], F32, name=f"gp_{half}_{hm}_{sc}",
                                          tag="gu", bufs=4)
                            up = psc.tile([128, 512], F32, name=f"up_{half}_{hm}_{sc}",
                                          tag="gu", bufs=4)
                            for k in range(NT):
                                mm(gp, wgub[:, k, 0:128], tin[k][:, sc * 512:(sc + 1) * 512],
                                   start=(k == 0), stop=(k == NT - 1))
                            for k in range(NT):
                                mm(up, wgub[:, k, 128:256], tin[k][:, sc * 512:(sc + 1) * 512],
                                   start=(k == 0), stop=(k == NT - 1))
                            sg = pcn.tile([128, 512], F32R, name=f"sg_{half}_{hm}_{sc}",
                                          tag="sg", bufs=2)
                            nc.scalar.activation(sg, gp, AF.Silu)
                            nc.vector.tensor_mul(F_t[sc][hm], sg, up)
                    # down-proj + residual + RMSNorm2
                    s2l = [pcn.tile([128, 1024], BF16, name=f"s2_{half}_{m}", tag=f"s2_{m}")
                           for m in range(NT)]
                    ssp2 = [psc.tile([1, 512], F32, name=f"ssp2_{half}_{sc}", tag="st", bufs=2)
                            for sc in range(2)]
                    sqt2 = [[None] * 2 for _ in range(NT)]
                    for m in range(NT):
                        wdb = wc.tile([128, NH, 128], BF16, name=f"wdb_{half}_{m}",
                                      tag="wd", bufs=3)
                        nc.sync.dma_start(wdb, wd4[:, m])
                        for sc in range(2):
                            dp = psc.tile([128, 512], F32, name=f"dp_{half}_{m}_{sc}",
                                          tag="d", bufs=2)
                            for hm in range(NH):
                                mm(dp, wdb[:, hm, :], F_t[sc][hm],
                                   start=(hm == 0), stop=(hm == NH - 1))
                            sl = s2l[m][:, sc * 512:(sc + 1) * 512]
                            nc.vector.tensor_add(sl, tin[m][:, sc * 512:(sc + 1) * 512], dp)
                            sqt = pcn.tile([128, 512], BF16, name=f"sq2_{half}_{m}_{sc}",
                                           tag=f"sq2_{sc}", bufs=8)
                            nc.vector.tensor_mul(sqt, sl, sl)
                            sqt2[m][sc] = sqt
                    for sc in range(2):
                        for m in range(NT):
                            mm(ssp2[sc], onesKb, sqt2[m][sc], start=(m == 0), stop=(m == NT - 1))
                    for sc in range(2):
                        row2 = pcn.tile([1, 512], F32R, name=f"row2_{half}_{sc}", tag="row2", bufs=2)
                        nc.scalar.activation(row2, ssp2[sc], AF.Sqrt, scale=1.0 / WIDTH, bias=eps1)
                        with nc.allow_low_precision(reason="tf32 rstd"):
                            nc.vector.reciprocal(row2, row2)
                        bc2 = psc.tile([128, 512], F32, name=f"bc2_{half}_{sc}", tag="d", bufs=2)
                        mm(bc2, _r(ones1), _r(row2))
                        bc2_s = pcn.tile([128, 512], F32R, name=f"bc2s_{half}_{sc}",
                                         tag="bc2s", bufs=2)
                        nc.vector.tensor_copy(bc2_s, bc2)
                        for m in range(NT):
                            ot = pcn.tile([128, 512], F32R, name=f"ot_{half}_{sc}_{m}",
                                          tag="ot", bufs=2)
                            nc.vector.scalar_tensor_tensor(ot, s2l[m][:, sc * 512:(sc + 1) * 512],
                                                           g2[:, m:m + 1], bc2_s,
                                                           op0=OP.mult, op1=OP.mult)
                            nc.sync.dma_start(
                                out_r[m][:, half * 1024 + sc * 512:half * 1024 + (sc + 1) * 512], ot)


_INPUT_SPECS = [
    ("xw_t", [WIDTH, LWIN], BF16),
    ("xq_t", [WIDTH, LQ], BF16),
    ("xr_t", [WIDTH, REST], BF16),
    ("wqk4", [128, NT, NT, 256], BF16),
    ("wv4", [128, 4, NT, 256], BF16),
    ("wo4", [128, NT, NT, 128], BF16),
    ("wgu4", [128, NH, NT, 256], BF16),
    ("wd4", [128, NT, NH, 128], BF16),
    ("cos_q", [128, LQ], BF16),
    ("sin_q", [128, LQ], BF16),
    ("cos_k", [128, LWIN], BF16),
    ("sin_k", [128, LWIN], BF16),
    ("g1", [128, NT], F32R),
    ("g2", [128, NT], F32R),
]


def build_program(reps=1):
    nc = bacc.Bacc("TRN2", target_bir_lowering=False, debug=False, num_devices=N_CORES)
    A = {name: nc.dram_tensor(name, shape, dt, kind="ExternalInput").ap()
         for name, shape, dt in _INPUT_SPECS}
    out_ap = nc.dram_tensor("out_t", [WIDTH, TOUT], F32R, kind="ExternalOutput").ap()
    with tile.TileContext(nc) as tc:
        for _ in range(reps):
            _emit(tc, A, out_ap)
    nc.compile()
    return nc


def _w4(w_t, a, c):
    """[K, M] weight (contract-major) -> [128, M/c-tiles, K/128, c] bf16 blocks."""
    k, m = w_t.shape
    return np.ascontiguousarray(
        w_t.reshape(k // 128, 128, m // c, c).transpose(1, 2, 0, 3)
        .astype(ml_dtypes.bfloat16))


def make_in_maps(x, w_qkv, w_out, g_norm1, g_norm2, w_gate, w_up, w_down):
    f32 = np.float32
    bf16 = ml_dtypes.bfloat16
    x = np.asarray(x, f32)
    w_qkv = np.asarray(w_qkv, f32)
    # head-contiguous de-interleave: within head h, even dims first then odd:
    # new row h*64+j -> old h*64+2j ; new row h*64+32+j -> old h*64+2j+1
    perm = np.empty(WIDTH, np.int64)
    for h in range(HEADS):
        j = np.arange(32)
        perm[h * 64 + j] = h * 64 + 2 * j
        perm[h * 64 + 32 + j] = h * 64 + 2 * j + 1
    wq = w_qkv[0:WIDTH][perm]
    wk = w_qkv[WIDTH:2 * WIDTH][perm]
    wv = w_qkv[2 * WIDTH:3 * WIDTH]

    inv_freq = (1.0 / (ROPE_BASE ** (np.arange(0, HDIM, 2, dtype=np.float64) / HDIM)))

    def tab(pos):
        # cos straight; sin with the rotation sign baked in per 32-row block
        # (rows r with r%64<32 hold even dims E: dest_E = E*cos - O*sin).
        fr = np.outer(inv_freq, pos.astype(np.float64))  # [32, T]
        s = np.sin(fr)
        return (np.tile(np.cos(fr), (4, 1)).astype(bf16),
                np.ascontiguousarray(
                    np.concatenate([-s, s, -s, s], axis=0)).astype(bf16))

    cos_k, sin_k = tab(np.arange(LWIN))
    wq4 = _w4(np.ascontiguousarray(wq.T), NT, 128)
    wk4 = _w4(np.ascontiguousarray(wk.T), NT, 128)
    wg4 = _w4(np.ascontiguousarray(np.asarray(w_gate, f32).T), NH, 128)
    wu4 = _w4(np.ascontiguousarray(np.asarray(w_up, f32).T), NH, 128)
    common = {
        "wqk4": np.ascontiguousarray(np.concatenate([wq4, wk4], axis=3)),
        "wv4": _w4(np.ascontiguousarray(wv.T), 4, 256),
        "wo4": _w4(np.ascontiguousarray(np.asarray(w_out, f32).T), NT, 128),
        "wgu4": np.ascontiguousarray(np.concatenate([wg4, wu4], axis=3)),
        "wd4": _w4(np.ascontiguousarray(np.asarray(w_down, f32).T), NT, 128),
        "cos_k": cos_k,
        "sin_k": sin_k,
        "g1": np.ascontiguousarray(np.asarray(g_norm1, f32).reshape(NT, 128).T),
        "g2": np.ascontiguousarray(np.asarray(g_norm2, f32).reshape(NT, 128).T),
    }
    in_maps = []
    for c in range(N_CORES):
        b, qh = c // 2, c % 2
        cos_q, sin_q = tab(np.arange(qh * LQ, (qh + 1) * LQ))
        m = dict(common)
        m["xw_t"] = np.ascontiguousarray(x[b, :LWIN].T).astype(bf16)
        m["xq_t"] = np.ascontiguousarray(x[b, qh * LQ:(qh + 1) * LQ].T).astype(bf16)
        m["xr_t"] = np.ascontiguousarray(
            x[b, LWIN + qh * REST:LWIN + (qh + 1) * REST].T).astype(bf16)
        m["cos_q"] = cos_q
        m["sin_q"] = sin_q
        in_maps.append(m)
    return in_maps


def assemble_output(results):
    out = np.empty((4, 4096, WIDTH), np.float32)
    for c in range(N_CORES):
        b, qh = c // 2, c % 2
        o = results[c]["out_t"]
        out[b, qh * LQ:(qh + 1) * LQ] = o[:, :LQ].T
        out[b, LWIN + qh * REST:LWIN + (qh + 1) * REST] = o[:, LQ:].T
    return out


_CACHE = {}


def kernel(x, w_qkv, w_out, g_norm1, g_norm2, w_gate, w_up, w_down):
    if "nc" not in _CACHE:
        _CACHE["nc"] = build_program()
    nc = _CACHE["nc"]
    in_maps = make_in_maps(x, w_qkv, w_out, g_norm1, g_norm2, w_gate, w_up, w_down)
    res = run_bass_kernel_spmd(nc, in_maps, list(range(N_CORES))).results
    return assemble_output(res)
